# revision 5
# baseline (speedup 1.0000x reference)
"""Trainium2 Bass kernel for HARSpikingNet (spiking MLP with LIF scan).

Reference computation:
    h_all = einsum('tbd,hd->tbh', x_spikes, W_in) + b_in     # [T,B,H] big GEMM
    scan over t: v = 0.9*v + h_t; z = (v > 1); v = v - z     # LIF with soft reset
    logits = z_last @ W_out.T + b_out                        # [B,C]

Strategy (8 cores, data-parallel over B):
  * Each core owns 64 batch rows.  The full fp32 GEMM precision is obtained
    from three fp16 matmul passes (x = x_hi + x_lo, W = W_hi + W_lo; the
    lo*lo term is below fp32 rounding) at 1 cycle/row on the PE, i.e. 3x
    cheaper than native fp32 matmul (4 cycles/row).  Spiking thresholds make
    bf16/fp32r matmuls numerically unusable (spike flips), measured.
  * Host pre-transposes x to x^T [D, T*64] per core so the contraction dim d
    lands on SBUF partitions with contiguous DMA; W^T likewise.
  * On-chip pipeline per 8-timestep super-chunk and per H-quarter (4 h-tiles
    of 128): PE accumulates h into PSUM (12 matmuls per 128x256 region:
    4 d-tiles x 3 split passes), ACT adds b_in (per-partition bias), DVE
    runs the LIF scan reading h straight from PSUM: 2 fused
    scalar_tensor_tensor ops per step:
        v   = (w * -beta) + h_t        # w == -v_post carries the sign trick
        w   = (v > theta) - v          # = z - v = -(v - z)
  * z_last (exact 0/1 in fp16) feeds the tiny output GEMM on-chip; b_out is
    added on the host (pure affine shift of the output).
"""

import numpy as np

T, B, D, H, C = 128, 512, 512, 2048, 18
NCORES = 8
BL = B // NCORES          # 64 batch rows per core
TB = T * BL               # 8192 (t*BL + b) columns per core
BETA, THETA = 0.9, 1.0

NS = 16                   # super-chunks of 8 timesteps (512 tb columns)
NQ = 4                    # H quarters, each 4 h-tiles of 128
ND = 4                    # d (contraction) tiles of 128
KT = H // 128             # 16 h-tiles


def _build_program():
    import concourse.bass as bass
    import concourse.bacc as bacc
    import concourse.tile as tile
    import concourse.mybir as mybir

    dt = mybir.dt
    op = mybir.AluOpType
    # Bacc (not bare Bass): its compile() pipeline runs
    # generate_event_semaphores, which legalizes multi-sync-wait
    # instructions (TRN2 allows 1 wait per instruction).
    nc = bacc.Bacc("TRN2", target_bir_lowering=False, debug=False,
                   enable_asserts=False)

    xt_hi = nc.declare_dram_parameter("xt_hi", [D, TB], dt.float16, isOutput=False)
    xt_lo = nc.declare_dram_parameter("xt_lo", [D, TB], dt.float16, isOutput=False)
    wt_hi = nc.declare_dram_parameter("wt_hi", [D, H], dt.float16, isOutput=False)
    wt_lo = nc.declare_dram_parameter("wt_lo", [D, H], dt.float16, isOutput=False)
    b_col = nc.declare_dram_parameter("b_col", [128, KT], dt.float32, isOutput=False)
    wout_hi = nc.declare_dram_parameter("wout_hi", [128, KT * C], dt.float16, isOutput=False)
    wout_lo = nc.declare_dram_parameter("wout_lo", [128, KT * C], dt.float16, isOutput=False)
    out_d = nc.declare_dram_parameter("out", [BL, C], dt.float32, isOutput=True)

    with tile.TileContext(nc) as tc:
        with (
            tc.tile_pool(name="const", bufs=1) as cpool,
            tc.tile_pool(name="xin", bufs=3) as xpool,
            tc.tile_pool(name="state", bufs=1) as spool,
            tc.tile_pool(name="hps", bufs=2, space=bass.MemorySpace.PSUM) as hpool,
            tc.tile_pool(name="outp", bufs=1) as opool,
        ):
            # ---- resident constants ----
            w_sb = {}
            for d in range(ND):
                for nm, dram in (("hi", wt_hi), ("lo", wt_lo)):
                    wt = cpool.tile([128, H], dt.float16, tag=f"w{d}{nm}")
                    nc.sync.dma_start(wt[:], dram[d * 128:(d + 1) * 128, :])
                    w_sb[(d, nm)] = wt
            bcol = cpool.tile([128, KT], dt.float32, tag="bcol")
            nc.sync.dma_start(bcol[:], b_col[:])
            wout_sb = {}
            for nm, dram in (("hi", wout_hi), ("lo", wout_lo)):
                wo = cpool.tile([128, KT * C], dt.float16, tag=f"wo{nm}")
                nc.sync.dma_start(wo[:], dram[:])
                wout_sb[nm] = wo

            # ACT warm-up: sync the scalar engine on the constant DMAs here so
            # the per-tile bias ops later carry a single (PE) sync wait —
            # walrus rejects ACT instructions with more than one wait.
            act_scratch = cpool.tile([128, 1], dt.float32, tag="ascr")
            nc.scalar.copy(act_scratch[:], bcol[:, 0:1])

            # ---- LIF state: free layout (h_tile 16, b 64); w = -v_post ----
            v = spool.tile([128, KT, BL], dt.float32, tag="v")
            z = spool.tile([128, KT, BL], dt.float16, tag="z")
            nc.vector.memset(v[:], 0.0)

            for s in range(NS):
                xs = {}
                for d in range(ND):
                    for nm, dram in (("hi", xt_hi), ("lo", xt_lo)):
                        xtile = xpool.tile([128, 512], dt.float16, tag=f"x{d}{nm}")
                        nc.sync.dma_start(
                            xtile[:],
                            dram[d * 128:(d + 1) * 128, s * 512:(s + 1) * 512],
                        )
                        xs[(d, nm)] = xtile

                for q in range(NQ):
                    ps = hpool.tile([128, 4, 512], dt.float32, tag="hps")
                    for i in range(4):
                        k = q * 4 + i
                        for half in range(2):
                            cnt = 0
                            for d in range(ND):
                                for wnm, xnm in (("hi", "hi"), ("hi", "lo"), ("lo", "hi")):
                                    nc.tensor.matmul(
                                        ps[:, i, half * 256:(half + 1) * 256],
                                        w_sb[(d, wnm)][:, k * 128:(k + 1) * 128],
                                        xs[(d, xnm)][:, half * 256:(half + 1) * 256],
                                        start=(cnt == 0),
                                        stop=(cnt == 11),
                                    )
                                    cnt += 1
                        # h += b_in (per-partition bias on the scalar engine)
                        nc.scalar.add(ps[:, i, :], ps[:, i, :], bcol[:, k:k + 1])

                    vq = v[:, q * 4:(q + 1) * 4, :]
                    for ti in range(8):
                        hcols = ps[:, :, ti * BL:(ti + 1) * BL]
                        nc.vector.scalar_tensor_tensor(
                            vq, vq, -BETA, hcols, op.mult, op.add
                        )
                        if s == NS - 1 and ti == 7:
                            nc.vector.tensor_scalar(
                                z[:, q * 4:(q + 1) * 4, :], vq, THETA, None, op.is_gt
                            )
                        else:
                            nc.vector.scalar_tensor_tensor(
                                vq, vq, THETA, vq, op.is_gt, op.subtract
                            )

            # ---- logits = z_last @ W_out.T  (b_out added on host) ----
            lp = hpool.tile([BL, C], dt.float32, tag="hps")
            cnt = 0
            for k in range(KT):
                for nm in ("hi", "lo"):
                    nc.tensor.matmul(
                        lp[:],
                        z[:, k, :],
                        wout_sb[nm][:, k * C:(k + 1) * C],
                        start=(cnt == 0),
                        stop=(cnt == 2 * KT - 1),
                    )
                    cnt += 1
            outt = opool.tile([BL, C], dt.float32, tag="out")
            nc.vector.tensor_copy(outt[:], lp[:])
            nc.sync.dma_start(out_d[:], outt[:])

    nc.compile()
    return nc


_NC_CACHE = None


def _get_nc():
    global _NC_CACHE
    if _NC_CACHE is None:
        _NC_CACHE = _build_program()
    return _NC_CACHE


def _split16(a):
    hi = a.astype(np.float16)
    lo = (a - hi.astype(np.float32)).astype(np.float16)
    return hi, lo


def kernel(x_spikes, W_in, b_in, W_out, b_out, _trace=False):
    x_spikes = np.asarray(x_spikes, dtype=np.float32)
    W_in = np.asarray(W_in, dtype=np.float32)
    b_in = np.asarray(b_in, dtype=np.float32)
    W_out = np.asarray(W_out, dtype=np.float32)
    b_out = np.asarray(b_out, dtype=np.float32)

    wt_hi, wt_lo = _split16(W_in.T.copy())                      # [D, H]
    b_col = b_in.reshape(KT, 128).T.copy()                      # [128, KT]
    # wout packed: [p, k*C + c] = W_out[c, k*128 + p]
    wout = W_out.T.reshape(KT, 128, C).transpose(1, 0, 2).reshape(128, KT * C)
    wo_hi, wo_lo = _split16(wout.copy())

    in_maps = []
    for c in range(NCORES):
        xc = x_spikes[:, c * BL:(c + 1) * BL, :]                # [T, BL, D]
        xt = xc.transpose(2, 0, 1).reshape(D, TB)               # [D, T*BL]
        x_hi, x_lo = _split16(xt)
        in_maps.append({
            "xt_hi": np.ascontiguousarray(x_hi),
            "xt_lo": np.ascontiguousarray(x_lo),
            "wt_hi": wt_hi, "wt_lo": wt_lo,
            "b_col": b_col,
            "wout_hi": wo_hi, "wout_lo": wo_lo,
        })

    from concourse.bass_utils import run_bass_kernel_spmd

    nc = _get_nc()
    res = run_bass_kernel_spmd(nc, in_maps, list(range(NCORES)), trace=_trace)
    out = np.concatenate([res.results[c]["out"] for c in range(NCORES)], axis=0)
    out = out + b_out[None, :]
    if _trace:
        kernel._last_results = res
    return out.astype(np.float32)
